# revision 28
# baseline (speedup 1.0000x reference)
"""Trainium2 Bass kernel for nn_CustomLSTM: B=32, S=512, D=512, H=1024.

Strategy (8 NeuronCores, one chip): TP-4 x DP-2.
  - The 8 cores split into two die-local quads ({0..3}, {4..7}); each quad
    handles half the batch (data parallel), so the per-step h all-gather
    never crosses the die-to-die link.
  - Within a quad, core position p owns hidden units [256p, 256p+256) and
    computes all four gates for those units (tensor parallel).
  - Phase 1: input projections xpre = x @ W_i* as W-stationary GEMMs
    (units-on-partition), kept resident in SBUF as bf16.
  - Phase 2: 512 sequential steps. Per step, one PSUM bank holds
    [128, 8 m-tiles x 16 batch]: a single identity-fold matmul injects the
    precomputed xpre tile (start=True), then 64 accumulating matmuls add
    W_h^T @ h_{t-1} (16 per sender: 2 k-chunks x 8 m-tiles). ACT does one
    sigmoid over 3 gates + tanh(g) + tanh(c); DVE updates c and emits the
    bf16 h_send tile; gpsimd broadcasts h_send to the 3 quad peers only
    (self-chunks are read locally, no loopback DMA).
  - Raw bass: hand-scheduled engine programs with explicit semaphores.

Self-contained: hardcodes all shapes; host side shards/reassembles.
"""
import numpy as np
import ml_dtypes

import concourse.bacc as bacc
import concourse.mybir as mybir
from concourse import bass_utils

F32 = mybir.dt.float32
BF16 = mybir.dt.bfloat16

B, S, D, H = 32, 512, 512, 1024
NCORES = 8
NQ = 4                    # cores per quad
BH = B // 2               # batch per quad = 16
UC = H // NQ              # units per core = 256

import os
if os.environ.get("KERNEL_SIM_STEPS"):
    S = int(os.environ["KERNEL_SIM_STEPS"])
ROWS = S * BH             # phase-1 rows per core (s-major, b-minor)
NB = ROWS // 512          # phase-1 row blocks of 512 rows

_cache = {}


def _build(detect_races=True):
    nc = bacc.Bacc(None, target_bir_lowering=False, num_devices=NCORES,
                   detect_race_conditions=detect_races)

    # ---------------- DRAM I/O ----------------
    xT_d = nc.dram_tensor("xT", [128, 4, ROWS], BF16, kind="ExternalInput")
    w_in_d = nc.dram_tensor("w_in", [128, 4, 1024], BF16, kind="ExternalInput")
    w_rec_d = nc.dram_tensor("w_rec", [128, 8, 1024], BF16, kind="ExternalInput")
    bias_d = nc.dram_tensor("bias", [128, 8], F32, kind="ExternalInput")
    ident_d = nc.dram_tensor("ident", [128, 128], BF16, kind="ExternalInput")
    h_out_d = nc.dram_tensor("h_out", [S * 128, 32], BF16, kind="ExternalOutput")
    c_out_d = nc.dram_tensor("c_out", [S * 128, 32], F32, kind="ExternalOutput")

    # ---------------- SBUF ----------------
    xt_buf = nc.alloc_sbuf_tensor("xt_buf", [128, 2, 4, 512], BF16)
    w_in_sb = nc.alloc_sbuf_tensor("w_in_sb", [128, 4, 1024], BF16)
    w_rec_sb = nc.alloc_sbuf_tensor("w_rec_sb", [128, 8, 1024], BF16)
    bias_sb = nc.alloc_sbuf_tensor("bias_sb", [128, 8], F32)
    ident_sb = nc.alloc_sbuf_tensor("ident_sb", [128, 128], BF16)
    # xpre[p, s, m, b]: m-tile order [i0,i1,f0,f1,o0,o1,g0,g1]
    xpre_sb = nc.alloc_sbuf_tensor("xpre_sb", [128, S, 8, 16], BF16)
    # h tiles from quad peers: [parity, sender-pos, (chunk, batch)]
    hT_buf = nc.alloc_sbuf_tensor("hT_buf", [128, 2, 4, 32], BF16)
    h_send = nc.alloc_sbuf_tensor("h_send", [128, 2, 32], BF16)
    gact = nc.alloc_sbuf_tensor("gact", [128, 4, 32], F32)   # i, f, o, g
    tmp1 = nc.alloc_sbuf_tensor("tmp1", [128, 32], F32)
    c_sb = nc.alloc_sbuf_tensor("c_sb", [128, 2, 32], F32)
    tanh_c = nc.alloc_sbuf_tensor("tanh_c", [128, 32], F32)

    pp = nc.alloc_psum_tensor("pp", [128, 8, 512], F32)

    # ---------------- semaphores ----------------
    arr = [nc.alloc_semaphore(f"arr{j}") for j in range(NQ)]
    loc_sem = nc.alloc_semaphore("loc")
    prep_sem = nc.alloc_semaphore("prep")
    dma_w = nc.alloc_semaphore("dma_w")
    xt_sem = nc.alloc_semaphore("xt_sem")
    p1_bank = nc.alloc_semaphore("p1_bank")
    p1fD = nc.alloc_semaphore("p1fD")
    p1fA = nc.alloc_semaphore("p1fA")
    psum_ready = nc.alloc_semaphore("psum_ready")
    act_g = nc.alloc_semaphore("act_g")
    c_ready = nc.alloc_semaphore("c_ready")
    tc_ready = nc.alloc_semaphore("tc_ready")
    h_ready = nc.alloc_semaphore("h_ready")
    outc_sem = nc.alloc_semaphore("outc_sem")
    outh_sem = nc.alloc_semaphore("outh_sem")

    xT = xT_d.ap()
    w_in = w_in_sb.ap()
    w_rec = w_rec_sb.ap()
    xpre = xpre_sb.ap()
    ppa = pp.ap()
    hT = hT_buf.ap()

    with nc.Block() as block:

        # ================= SP: DMA feeder + output writer =================
        @block.sync
        def _(sp):
            sp.dma_start(w_in_sb.ap(), w_in_d.ap()).then_inc(dma_w, 16)
            sp.dma_start(w_rec_sb.ap(), w_rec_d.ap()).then_inc(dma_w, 16)
            sp.dma_start(ident_sb.ap(), ident_d.ap()).then_inc(dma_w, 16)
            sp.dma_start(bias_sb.ap(), bias_d.ap()).then_inc(dma_w, 16)
            for n in range(NB):
                if n >= 2:
                    sp.wait_ge(p1_bank, 8 * (n - 1))
                sp.dma_start(
                    xt_buf.ap()[:, n % 2, :, :], xT[:, :, n * 512:(n + 1) * 512]
                ).then_inc(xt_sem, 16)
            # phase 2 outputs. Deferred until the step's broadcast frame has
            # been sent: output writes share the 16 SDMA engines with the
            # broadcast, and firing them first delays the latency-critical
            # h exchange.
            for t in range(S):
                sp.wait_ge(c_ready, t + 1)
                sp.wait_ge(h_ready, t + 1)
                sp.wait_ge(loc_sem, 16 * min(t + 1, S - 1))
                sp.dma_start(
                    h_out_d.ap()[t * 128:(t + 1) * 128, :], h_send.ap()[:, t % 2, :]
                ).then_inc(outh_sem, 16)
                sp.dma_start(
                    c_out_d.ap()[t * 128:(t + 1) * 128, :], c_sb.ap()[:, t % 2, :]
                ).then_inc(outc_sem, 16)

        # ================= PE =================
        @block.tensor
        def _(pe):
            pe.wait_ge(dma_w, 64)
            # ---- phase 1: xpre^T = W_in^T @ x^T, units on partitions ----
            for n in range(NB):
                pe.wait_ge(xt_sem, 16 * (n + 1))
                for m in range(8):
                    if n >= 1:
                        if m < 4:
                            pe.wait_ge(p1fD, 4 * (n - 1) + m + 1)
                        else:
                            pe.wait_ge(p1fA, 4 * (n - 1) + (m - 4) + 1)
                    for k in range(4):
                        ins = nc.tensor.matmul(
                            ppa[:, m, 0:512],
                            w_in[:, k, m * 128:(m + 1) * 128],
                            xt_buf.ap()[:, n % 2, k, :],
                            start=(k == 0),
                            stop=(k == 3),
                        )
                    ins.then_inc(p1_bank, 1)
            # ---- phase 2 ----
            pe.wait_ge(p1fD, 4 * NB)
            pe.wait_ge(p1fA, 4 * NB)
            for t in range(S):
                p = t % 2
                if t >= 2:
                    pe.wait_ge(act_g, 2 * t - 2)
                # xpre fold-in: one matmul, clears bank (start=True)
                ins = nc.tensor.matmul(
                    ppa[:, p, 0:128],
                    ident_sb.ap(),
                    xpre[:, t, :, :],
                    start=True,
                    stop=(t == 0),
                )
                if t >= 1:
                    pq = (t - 1) % 2
                    # all 4 quad sender slots (incl. own loopback) land in hT
                    for sidx in range(NQ):
                        pe.wait_ge(arr[sidx], 2 * t)
                        for m in range(8):
                            ins = nc.tensor.matmul(
                                ppa[:, p, m * 16:(m + 1) * 16],
                                w_rec[:, 2 * sidx, m * 128:(m + 1) * 128],
                                hT[:, pq, sidx, 0:16],
                                start=False,
                                stop=False,
                            )
                        for m in range(8):
                            last = (sidx == NQ - 1)
                            ins = nc.tensor.matmul(
                                ppa[:, p, m * 16:(m + 1) * 16],
                                w_rec[:, 2 * sidx + 1, m * 128:(m + 1) * 128],
                                hT[:, pq, sidx, 16:32],
                                start=False,
                                stop=last,
                            )
                ins.then_inc(psum_ready, 1)

        # ================= ACT =================
        @block.scalar
        def _(act):
            # phase 1 evacuation (m-tiles 4..7): psum -> xpre (bf16) + bias
            for n in range(NB):
                for m in range(4, 8):
                    act.wait_ge(p1_bank, 8 * n + m + 1)
                    nc.scalar.activation(
                        xpre[:, n * 32:(n + 1) * 32, m, :],
                        ppa[:, m, 0:512],
                        mybir.ActivationFunctionType.Identity,
                        bias=bias_sb.ap()[:, m:m + 1],
                    ).then_inc(p1fA, 1)
            for t in range(S):
                p = t % 2
                act.wait_ge(psum_ready, t + 1)
                # sigmoid over gates i, f, o (psum cols 0:96)
                nc.scalar.activation(
                    gact.ap()[:, 0:3, :],
                    ppa[:, p, 0:96],
                    mybir.ActivationFunctionType.Sigmoid,
                ).then_inc(act_g, 1)
                nc.scalar.activation(
                    gact.ap()[:, 3, :],
                    ppa[:, p, 96:128],
                    mybir.ActivationFunctionType.Tanh,
                ).then_inc(act_g, 1)
                act.wait_ge(c_ready, t + 1)
                nc.scalar.activation(
                    tanh_c.ap(),
                    c_sb.ap()[:, p, :],
                    mybir.ActivationFunctionType.Tanh,
                ).then_inc(tc_ready, 1)

        # ================= DVE =================
        @block.vector
        def _(dve):
            dve.memset(c_sb.ap()[:, 1, :], 0.0)
            dve.drain()
            # phase 1 evacuation (m-tiles 0..3): psum -> xpre (bf16) + bias
            for n in range(NB):
                for m in range(4):
                    dve.wait_ge(p1_bank, 8 * n + m + 1)
                    nc.vector.tensor_scalar_add(
                        xpre[:, n * 32:(n + 1) * 32, m, :],
                        ppa[:, m, 0:512],
                        bias_sb.ap()[:, m:m + 1],
                    ).then_inc(p1fD, 1)
            # phase 2
            for t in range(S):
                p = t % 2
                # c = f * c_prev + i * g
                dve.wait_ge(act_g, 2 * t + 1)
                if t >= 2:
                    dve.wait_ge(outc_sem, 16 * (t - 1))
                nc.vector.tensor_mul(
                    c_sb.ap()[:, p, :], gact.ap()[:, 1, :], c_sb.ap()[:, 1 - p, :]
                )
                dve.wait_ge(act_g, 2 * t + 2)
                nc.vector.tensor_mul(tmp1.ap(), gact.ap()[:, 0, :], gact.ap()[:, 3, :])
                # distance-1 RAW on the DVE pipe needs an explicit drain
                dve.drain()
                nc.vector.tensor_add(
                    c_sb.ap()[:, p, :], c_sb.ap()[:, p, :], tmp1.ap()
                ).then_inc(c_ready, 1)
                # h = o * tanh(c), bf16, feeds broadcast + HBM output. The
                # own hT slot is written by gpsimd (per-core branch there).
                dve.wait_ge(tc_ready, t + 1)
                if t >= 2:
                    dve.wait_ge(outh_sem, 16 * (t - 1))
                    dve.wait_ge(loc_sem, 16 * (t - 1))
                nc.vector.tensor_mul(
                    h_send.ap()[:, p, :], gact.ap()[:, 2, :], tanh_c.ap()
                ).then_inc(h_ready, 1)

        # ================= Pool: quad broadcast =================
        @block.gpsimd
        def _(g):
            g.bir_kernel_barrier_wait([list(range(NCORES))])
            pid_reg = g.to_reg(g.partition_id())
            for kcore in range(NCORES):
                pos = kcore % NQ
                with g.If_eq(pid_reg, kcore):
                    for t in range(S - 1):
                        if t >= 1:
                            # descriptor-carveout reclaim
                            g.wait_ge(loc_sem, 16 * t)
                        g.remote_dma_broadcast(
                            out_ap=hT[:, t % 2, pos, :],
                            in_ap=h_send.ap()[:, t % 2, :],
                            remote_sem=arr[pos],
                            local_sem=loc_sem,
                            rdests=[(0, j) for j in range(NQ)]
                            + [None] * (8 - NQ),
                        ).then_inc(prep_sem, 1)
                        g.wait_ge(prep_sem, t + 1)
                        g.wait_ge(h_ready, t + 1)
                        g.trigger_dma(1)

    nc.finalize()
    return nc


def _prep_inputs(x, W_ii, W_if, W_ig, W_io, W_hi, W_hf, W_hg, W_ho,
                 b_i, b_f, b_g, b_o):
    bf = ml_dtypes.bfloat16
    ident = np.eye(128, dtype=bf)

    # per batch-half xT: [D, S, BH] -> [4, 128, S*BH] -> [128, 4, S*BH]
    xTs = []
    for half in range(2):
        xh = x[BH * half:BH * (half + 1)]           # [16, S, D]
        xT = np.ascontiguousarray(
            xh.transpose(2, 1, 0).reshape(4, 128, ROWS).transpose(1, 0, 2)
        ).astype(bf)
        xTs.append(xT)

    in_maps = []
    for c in range(NCORES):
        pos = c % NQ
        half = c // NQ
        U = slice(UC * pos, UC * (pos + 1))
        # m-tile order [i0,i1,f0,f1,o0,o1,g0,g1]
        w_in_c = np.concatenate(
            [W_ii[:, U], W_if[:, U], W_io[:, U], W_ig[:, U]], axis=1
        )  # [512, 1024]
        w_in_c = w_in_c.reshape(4, 128, 1024).transpose(1, 0, 2).astype(bf)
        w_rec_c = np.concatenate(
            [W_hi[:, U], W_hf[:, U], W_ho[:, U], W_hg[:, U]], axis=1
        )  # [1024, 1024]
        w_rec_c = w_rec_c.reshape(8, 128, 1024).transpose(1, 0, 2).astype(bf)
        bias_c = np.concatenate(
            [b_i[U], b_f[U], b_o[U], b_g[U]]
        ).reshape(8, 128).T.astype(np.float32)  # [128, 8]
        in_maps.append({
            "xT": xTs[half],
            "w_in": np.ascontiguousarray(w_in_c),
            "w_rec": np.ascontiguousarray(w_rec_c),
            "bias": np.ascontiguousarray(bias_c),
            "ident": ident,
        })
    return in_maps


def run(inputs, trace=False):
    if "nc" not in _cache:
        _cache["nc"] = _build()
    nc = _cache["nc"]
    in_maps = _prep_inputs(**inputs)
    res = bass_utils.run_bass_kernel_spmd(
        nc, in_maps, core_ids=list(range(NCORES)), trace=trace,
    )
    outputs = np.empty((B, S, H), np.float32)
    cells = np.empty((B, S, H), np.float32)
    for c in range(NCORES):
        pos = c % NQ
        half = c // NQ
        U = UC * pos
        Bsl = slice(BH * half, BH * (half + 1))
        # h_out rows t*128+p, cols (chunk, b): unit = U + 128*chunk + p
        h = res.results[c]["h_out"].astype(np.float32).reshape(S, 128, 2, 16)
        cc = res.results[c]["c_out"].reshape(S, 128, 2, 16)
        for chunk in range(2):
            Us = slice(U + 128 * chunk, U + 128 * (chunk + 1))
            outputs[Bsl, :, Us] = h[:, :, chunk, :].transpose(2, 0, 1)
            cells[Bsl, :, Us] = cc[:, :, chunk, :].transpose(2, 0, 1)
    return (outputs, cells), res


def kernel(**inputs):
    (outputs, cells), _ = run(inputs, trace=False)
    return outputs, cells
